# revision 10
# baseline (speedup 1.0000x reference)
"""PIoU (pixel-wise IoU) pairwise matrix kernel for Trainium2, 8 NeuronCores.

Math: for each pair (predicted box n, target box m) the reference samples a
16x16 grid of the joint AABB and evaluates a soft membership
F = sigmoid(k(w/2-|A|)) * sigmoid(k(h/2-|B|)) per box, where (A, B) are the
pixel offsets rotated into the box frame.  A and B are *affine* in the grid
coordinates (ug, uh), so k*(A, B) for all 256 pixels x 4 fields comes from
ONE K=12 matmul per 128-pair tile against a constant [1, ug, uh] basis;
the k*s/2 offset folds into the sigmoid's per-partition bias:
    F-factor = sigmoid(-|k*d| + k*s/2)

NMS gating (the big one): with k=10 the sigmoid tails die within ~1px, so
any pair whose dilated AABBs (delta=2px) do not overlap has
piou < 1e-14 -- indistinguishable from 0 at fp32.  Only ~8% of the 512x512
pairs survive.  The host computes the O(N*M) AABB overlap mask (cheap
numpy), gathers the active pairs into 128-wide tiles (partition = pair),
and scatters the device results back into the zero matrix.  Each core gets
the active pairs of its 64 predicted boxes (~21 tiles); all cores are
padded to the same tile count T with duplicate pairs so one SPMD program
serves all 8.

Per 128-pair tile (pair quantities all live per-partition, so every
sigmoid bias is a [128,1] AP):
  PE      [128,32]->[32,128] coefficient transpose + 2 fp16 matmuls
          (512 cols each) -> k*d in [128,1024] fp32 PSUM
  DVE     |k*d| via size-1-axis reduce with apply_absolute_value
          (PSUM->SBUF fp16), then fused product+pixel-sum
          scalar_tensor_tensor -> Ssum, and the Isum reduce
  ACT     lhsT copy + 4x sigmoid(scale=-1, bias=k*s_f/2)
  GPSIMD  F12 = F1*F2 product
Ssum/Isum land in column t of [128,T] accumulators; the epilogue computes
piou = I/(S-I+eps) for all tiles and DMAs one [128,T] block out.
"""

import numpy as np

N = 512
M = 512
G = 16
NPIX = G * G
K_SLOPE = np.float32(10.0)
EPS = np.float32(1e-6)
NC = 8
NLOC = N // NC  # 64 predicted boxes per core
DELTA = np.float32(2.0)  # AABB dilation for the gating mask (px)

_cache = {}

# gathered per-pair quantity order: P-side 0..9, T-side 10..19
QORD = ("x0", "x1", "y0", "y1", "cx", "cy", "ct", "st", "khw", "khh")


def _derived(b, k):
    # b: [K,5] float32 -> per-box derived quantities, coords pre-scaled by k
    cx, cy, w, h, t = (b[:, i].astype(np.float32) for i in range(5))
    c, s = np.cos(t).astype(np.float32), np.sin(t).astype(np.float32)
    hw = np.float32(0.5) * (w * np.abs(c) + h * np.abs(s))
    hh = np.float32(0.5) * (w * np.abs(s) + h * np.abs(c))
    return dict(
        cx=k * cx, cy=k * cy, ct=c, st=s,
        khw=(k * np.float32(0.5)) * w, khh=(k * np.float32(0.5)) * h,
        x0=k * (cx - hw), x1=k * (cx + hw), y0=k * (cy - hh), y1=k * (cy + hh),
    )


def _basis():
    u = ((np.arange(G, dtype=np.float32) + np.float32(0.5)) / np.float32(G))
    Ug = np.tile(u, G)      # pixel p = h*G+g -> u[g]
    Uh = np.repeat(u, G)    # -> u[h]
    # [12, 1024]: field f (A1,A2,B1,B2) rows 3f..3f+2 = 1,Ug,Uh on its cols
    basis = np.zeros((12, 4 * NPIX), dtype=np.float16)
    for f in range(4):
        c0 = f * NPIX
        basis[3 * f + 0, c0:c0 + NPIX] = 1.0
        basis[3 * f + 1, c0:c0 + NPIX] = Ug.astype(np.float16)
        basis[3 * f + 2, c0:c0 + NPIX] = Uh.astype(np.float16)
    # replicated into 32-partition strips 0/32/64 so lhsT slices for 3
    # consecutive tiles (one shared PE transpose) read rhs from their own
    # base partition
    BAS4 = np.zeros((128, 4 * NPIX), dtype=np.float16)
    for i in range(3):
        BAS4[32 * i:32 * i + 12, :] = basis
    return BAS4


def _active_pairs(loc_p, loc_t):
    """Per-core gathered pair lists from the dilated-AABB overlap mask."""
    P = _derived(loc_p, np.float32(1.0))
    T = _derived(loc_t, np.float32(1.0))
    ov = ((np.minimum(P["x1"][:, None], T["x1"][None, :]) + DELTA
           >= np.maximum(P["x0"][:, None], T["x0"][None, :])) &
          (np.minimum(P["y1"][:, None], T["y1"][None, :]) + DELTA
           >= np.maximum(P["y0"][:, None], T["y0"][None, :])))
    pairs = []
    for c in range(NC):
        n_idx, m_idx = np.nonzero(ov[c * NLOC:(c + 1) * NLOC])
        pairs.append((n_idx.astype(np.int64) + c * NLOC, m_idx.astype(np.int64)))
    ntiles = max(1, max((len(n) + 127) // 128 for n, _ in pairs))
    return pairs, ntiles


def _host_constants(loc_p, loc_t, pairs, T):
    """CQ [128, 20, T] per core: gathered, K-prescaled pair quantities."""
    Pq = _derived(loc_p, K_SLOPE)
    Tq = _derived(loc_t, K_SLOPE)
    CQs = []
    for c in range(NC):
        n_idx, m_idx = pairs[c]
        cnt = len(n_idx)
        pad = T * 128 - cnt
        if cnt == 0:
            n_idx = np.array([c * NLOC], np.int64)
            m_idx = np.array([0], np.int64)
            cnt, pad = 1, T * 128 - 1
        n_full = np.concatenate([n_idx, np.repeat(n_idx[:1], pad)])
        m_full = np.concatenate([m_idx, np.repeat(m_idx[:1], pad)])
        CQ = np.empty((20, T * 128), dtype=np.float32)
        for qi, q in enumerate(QORD):
            CQ[qi] = Pq[q][n_full]
            CQ[10 + qi] = Tq[q][m_full]
        # [20, T*128] -> [128, 20, T]  (pair j = t*128 + p)
        CQs.append(np.ascontiguousarray(
            CQ.reshape(20, T, 128).transpose(2, 0, 1)).reshape(128, 20 * T))
    return CQs


def _build_nc(T):
    from contextlib import ExitStack

    import concourse.bacc as bacc
    import concourse.tile as tile
    from concourse import mybir
    from concourse.masks import make_identity

    dt = mybir.dt
    op = mybir.AluOpType
    AF = mybir.ActivationFunctionType

    # Bacc (not raw Bass): its finalize() runs generate_event_semaphores,
    # which legalizes Tile's multi-wait sync_info down to <=1 wait per
    # hardware instruction.
    nc = bacc.Bacc(None, target_bir_lowering=False)
    CQ_d = nc.declare_dram_parameter("CQ", [128, 20 * T], dt.float32, isOutput=False)
    BAS_d = nc.declare_dram_parameter("BAS", [128, 4 * NPIX], dt.float16, isOutput=False)
    OUT_d = nc.declare_dram_parameter("OUT", [128, T], dt.float32, isOutput=True)

    with tile.TileContext(nc) as tc, ExitStack() as ctx:
        consts = ctx.enter_context(tc.tile_pool(name="consts", bufs=1))
        work = ctx.enter_context(tc.tile_pool(name="work", bufs=2))
        psum = ctx.enter_context(tc.tile_pool(name="psum", bufs=2, space="PSUM"))

        ident = consts.tile([128, 128], dt.float32)
        make_identity(nc, ident[:])
        CQ = consts.tile([128, 20, T], dt.float32)
        nc.sync.dma_start(out=CQ[:].rearrange("p a b -> p (a b)"), in_=CQ_d[:])
        BAS = consts.tile([128, 4 * NPIX], dt.float16)
        nc.sync.dma_start(out=BAS[:], in_=BAS_d[:])

        def q(i):
            return CQ[:, i, :]

        # ---- coefficient slab C [128 pair, T, 32 r] ----
        # rows r=0..11 = K-scaled affine coefficients (field f rows 3f..3f+2);
        # rows 12..31 junk padding (never fed to the matmul).
        C = consts.tile([128, T, 32], dt.float32)
        S = consts.tile([128, 12, T], dt.float32)
        g = nc.vector

        def s(i):
            return S[:, i, :]

        def c(r):
            return C[:, :, r]

        g.tensor_tensor(s(0), q(0), q(10), op.min)    # k*xmin
        g.tensor_tensor(s(1), q(1), q(11), op.max)    # k*xmax
        g.tensor_tensor(s(2), q(2), q(12), op.min)    # k*ymin
        g.tensor_tensor(s(3), q(3), q(13), op.max)    # k*ymax
        g.tensor_tensor(s(4), s(1), s(0), op.subtract)   # k*sx
        g.tensor_tensor(s(5), s(3), s(2), op.subtract)   # k*sy
        g.tensor_tensor(s(6), s(0), q(4), op.subtract)   # k*dxp
        g.tensor_tensor(s(7), s(2), q(5), op.subtract)   # k*dyp
        # A1 rows 0..2: k*(dxp*ctp + dyp*stp), k*sx*ctp, k*sy*stp
        g.tensor_tensor(s(8), s(6), q(6), op.mult)
        g.tensor_tensor(s(9), s(7), q(7), op.mult)
        g.tensor_tensor(c(0), s(8), s(9), op.add)
        g.tensor_tensor(c(1), s(4), q(6), op.mult)
        g.tensor_tensor(c(2), s(5), q(7), op.mult)
        # B1 rows 6..8: k*(dyp*ctp - dxp*stp), -k*sx*stp, k*sy*ctp
        g.tensor_tensor(s(8), s(7), q(6), op.mult)
        g.tensor_tensor(s(9), s(6), q(7), op.mult)
        g.tensor_tensor(c(6), s(8), s(9), op.subtract)
        g.scalar_tensor_tensor(c(7), s(4), -1.0, q(7), op.mult, op.mult)
        g.tensor_tensor(c(8), s(5), q(6), op.mult)
        # target box offsets
        g.tensor_tensor(s(10), s(0), q(14), op.subtract)  # k*dxt
        g.tensor_tensor(s(11), s(2), q(15), op.subtract)  # k*dyt
        # A2 rows 3..5
        g.tensor_tensor(s(8), s(10), q(16), op.mult)
        g.tensor_tensor(s(9), s(11), q(17), op.mult)
        g.tensor_tensor(c(3), s(8), s(9), op.add)
        g.tensor_tensor(c(4), s(4), q(16), op.mult)
        g.tensor_tensor(c(5), s(5), q(17), op.mult)
        # B2 rows 9..11
        g.tensor_tensor(s(8), s(11), q(16), op.mult)
        g.tensor_tensor(s(9), s(10), q(17), op.mult)
        g.tensor_tensor(c(9), s(8), s(9), op.subtract)
        g.scalar_tensor_tensor(c(10), s(4), -1.0, q(17), op.mult, op.mult)
        g.tensor_tensor(c(11), s(5), q(16), op.mult)

        Ssum = consts.tile([128, T], dt.float32)
        Isum = consts.tile([128, T], dt.float32)

        # ---- main loop over pair tiles ----
        for t in range(T):
            TP = psum.tile([128, 128], dt.float32, tag="tpose", bufs=2)
            if t == 0:
                # Warm the PE on the BAS DMA sem (single-wait) before the
                # first real matmul, which must wait on the ACT-written lhsT.
                # WAW into TP orders the real transpose after it.
                nc.tensor.matmul(
                    TP[:, 0:128], BAS[0:12, 0:128], BAS[0:12, 0:128],
                    start=True, stop=True)
            if True:
                nc.tensor.transpose(TP[0:32, 0:128], C[:, t, :], ident[:])
                lhsT = work.tile([32, 128], dt.float16, tag="lhsT", bufs=3)
                nc.scalar.copy(lhsT[:], TP[0:32, 0:128])

                i = 0
                lw = lhsT[0:12, :]
                F = psum.tile([128, 4 * NPIX], dt.float32, tag="fields", bufs=3)
                for qq in (0, 1):
                    nc.tensor.matmul(
                        F[:, qq * 512:(qq + 1) * 512],
                        lw, BAS[0:12, qq * 512:(qq + 1) * 512],
                        start=True, stop=True)
                # |k*d| : PSUM fp32 -> SBUF fp16 in one DVE pass (abs_max is
                # not ISA-legal in TensorScalar; a size-1-axis reduce with
                # apply_absolute_value is)
                absk = work.tile([128, 4 * NPIX], dt.float16, tag="absk", bufs=3)
                nc.vector.tensor_reduce(
                    absk[:], F[:].rearrange("p (c one) -> p c one", one=1),
                    mybir.AxisListType.X, op.max, apply_absolute_value=True)
                # sigmoid(k*s_f/2 - |k*d|) per field; bias = [128,1] AP
                sig = work.tile([128, 4 * NPIX], dt.float16, tag="sig", bufs=3)
                for f, bq in enumerate((8, 18, 9, 19)):  # khw_p khw_t khh_p khh_t
                    nc.scalar.activation(
                        sig[:, f * NPIX:(f + 1) * NPIX],
                        absk[:, f * NPIX:(f + 1) * NPIX],
                        AF.Sigmoid, bias=CQ[:, bq, t:t + 1], scale=-1.0)
                # Fp = [F1|F2] = [A1,A2]*[B1,B2]; Ssum[t] = sum(F1)+sum(F2)
                # (DVE scalar_tensor_tensor fuses product + pixel sum;
                # InstTensorTensorReduce crashes TRN2 hardware)
                Fp = work.tile([128, 2 * NPIX], dt.float16, tag="Fp", bufs=3)
                nc.vector.scalar_tensor_tensor(
                    Fp[:], sig[:, 0:2 * NPIX], 1.0, sig[:, 2 * NPIX:4 * NPIX],
                    op.mult, op.mult, accum_out=Ssum[:, t:t + 1])
                # F12 = F1*F2 on GPSIMD; Isum[t] = sum(F12) on DVE
                F12 = work.tile([128, NPIX], dt.float16, tag="F12", bufs=3)
                nc.gpsimd.tensor_tensor(
                    F12[:], Fp[:, 0:NPIX], Fp[:, NPIX:2 * NPIX], op.mult)
                nc.vector.tensor_reduce(
                    Isum[:, t:t + 1], F12[:], mybir.AxisListType.X, op.add)

        # ---- epilogue: piou = inter / (stot - inter + eps) ----
        union = consts.tile([128, T], dt.float32)
        nc.vector.scalar_tensor_tensor(
            union[:], Isum[:], -1.0, Ssum[:], op.mult, op.add)
        nc.vector.tensor_scalar(union[:], union[:], float(EPS), None, op.add)
        rec = consts.tile([128, T], dt.float32)
        nc.vector.reciprocal(rec[:], union[:])
        piou = consts.tile([128, T], dt.float32)
        nc.vector.tensor_tensor(piou[:], Isum[:], rec[:], op.mult)
        nc.sync.dma_start(out=OUT_d[:], in_=piou[:])

    nc.finalize()
    return nc


def _get_compiled(T):
    if T not in _cache:
        _cache[T] = _build_nc(T)
    return _cache[T]


def kernel(loc_p, loc_t, grid):
    from concourse.bass_utils import run_bass_kernel_spmd

    assert int(grid) == G
    loc_p = np.asarray(loc_p, dtype=np.float32)
    loc_t = np.asarray(loc_t, dtype=np.float32)
    pairs, T = _active_pairs(loc_p, loc_t)
    CQs = _host_constants(loc_p, loc_t, pairs, T)
    BAS = _basis()

    nc = _get_compiled(T)
    in_maps = [{"CQ": CQs[c], "BAS": BAS} for c in range(NC)]
    res = run_bass_kernel_spmd(nc, in_maps, core_ids=list(range(NC)))
    out = np.zeros((N, M), dtype=np.float32)
    for c in range(NC):
        n_idx, m_idx = pairs[c]
        cnt = len(n_idx)
        vals = res.results[c]["OUT"].T.reshape(-1)[:cnt]  # pair j = t*128+p
        out[n_idx, m_idx] = vals
    return out


# revision 11
# speedup vs baseline: 1.2228x; 1.2228x over previous
"""PIoU (pixel-wise IoU) pairwise matrix kernel for Trainium2, 8 NeuronCores.

Math: for each pair (predicted box n, target box m) the reference samples a
16x16 grid of the joint AABB and evaluates a soft membership
F = sigmoid(k(w/2-|A|)) * sigmoid(k(h/2-|B|)) per box, where (A, B) are the
pixel offsets rotated into the box frame.  A and B are *affine* in the grid
coordinates (ug, uh), so k*(A, B) for all 256 pixels x 4 fields comes from
ONE K=12 matmul per 128-pair tile against a constant [1, ug, uh] basis;
the k*s/2 offset folds into the sigmoid's per-partition bias:
    F-factor = sigmoid(-|k*d| + k*s/2)

NMS gating (the big one): with k=10 the sigmoid tails die within ~1px, so
any pair whose dilated AABBs (delta=2px) do not overlap has
piou < 1e-14 -- indistinguishable from 0 at fp32.  Only ~8% of the 512x512
pairs survive.  The host computes the O(N*M) AABB overlap mask (cheap
numpy), gathers the active pairs into 128-wide tiles (partition = pair),
and scatters the device results back into the zero matrix.  Each core gets
the active pairs of its 64 predicted boxes (~21 tiles); all cores are
padded to the same tile count T with duplicate pairs so one SPMD program
serves all 8.

Per 128-pair tile (pair quantities all live per-partition, so every
sigmoid bias is a [128,1] AP):
  PE      [128,32]->[32,128] coefficient transpose + 2 fp16 matmuls
          (512 cols each) -> k*d in [128,1024] fp32 PSUM
  DVE     |k*d| via size-1-axis reduce with apply_absolute_value
          (PSUM->SBUF fp16), then fused product+pixel-sum
          scalar_tensor_tensor -> Ssum, and the Isum reduce
  ACT     lhsT copy + 4x sigmoid(scale=-1, bias=k*s_f/2)
  GPSIMD  F12 = F1*F2 product
Ssum/Isum land in column t of [128,T] accumulators; the epilogue computes
piou = I/(S-I+eps) for all tiles and DMAs one [128,T] block out.
"""

import numpy as np

N = 512
M = 512
G = 16
NPIX = G * G
K_SLOPE = np.float32(10.0)
EPS = np.float32(1e-6)
NC = 8
NLOC = N // NC  # 64 predicted boxes per core
DELTA = np.float32(2.0)  # AABB dilation for the gating mask (px)

_cache = {}

# gathered per-pair quantity order: P-side 0..9, T-side 10..19
QORD = ("x0", "x1", "y0", "y1", "cx", "cy", "ct", "st", "khw", "khh")


def _derived(b, k):
    # b: [K,5] float32 -> per-box derived quantities, coords pre-scaled by k
    cx, cy, w, h, t = (b[:, i].astype(np.float32) for i in range(5))
    c, s = np.cos(t).astype(np.float32), np.sin(t).astype(np.float32)
    hw = np.float32(0.5) * (w * np.abs(c) + h * np.abs(s))
    hh = np.float32(0.5) * (w * np.abs(s) + h * np.abs(c))
    return dict(
        cx=k * cx, cy=k * cy, ct=c, st=s,
        khw=(k * np.float32(0.5)) * w, khh=(k * np.float32(0.5)) * h,
        x0=k * (cx - hw), x1=k * (cx + hw), y0=k * (cy - hh), y1=k * (cy + hh),
    )


def _basis():
    u = ((np.arange(G, dtype=np.float32) + np.float32(0.5)) / np.float32(G))
    Ug = np.tile(u, G)      # pixel p = h*G+g -> u[g]
    Uh = np.repeat(u, G)    # -> u[h]
    # [12, 1024]: field f (A1,A2,B1,B2) rows 3f..3f+2 = 1,Ug,Uh on its cols
    basis = np.zeros((12, 4 * NPIX), dtype=np.float16)
    for f in range(4):
        c0 = f * NPIX
        basis[3 * f + 0, c0:c0 + NPIX] = 1.0
        basis[3 * f + 1, c0:c0 + NPIX] = Ug.astype(np.float16)
        basis[3 * f + 2, c0:c0 + NPIX] = Uh.astype(np.float16)
    # replicated into 32-partition strips 0/32/64 so lhsT slices for 3
    # consecutive tiles (one shared PE transpose) read rhs from their own
    # base partition
    BAS4 = np.zeros((128, 4 * NPIX), dtype=np.float16)
    for i in range(3):
        BAS4[32 * i:32 * i + 12, :] = basis
    return BAS4


def _active_pairs(loc_p, loc_t):
    """Per-core gathered pair lists from the dilated-AABB overlap mask."""
    P = _derived(loc_p, np.float32(1.0))
    T = _derived(loc_t, np.float32(1.0))
    ov = ((np.minimum(P["x1"][:, None], T["x1"][None, :]) + DELTA
           >= np.maximum(P["x0"][:, None], T["x0"][None, :])) &
          (np.minimum(P["y1"][:, None], T["y1"][None, :]) + DELTA
           >= np.maximum(P["y0"][:, None], T["y0"][None, :])))
    pairs = []
    for c in range(NC):
        n_idx, m_idx = np.nonzero(ov[c * NLOC:(c + 1) * NLOC])
        pairs.append((n_idx.astype(np.int64) + c * NLOC, m_idx.astype(np.int64)))
    ntiles = max(1, max((len(n) + 127) // 128 for n, _ in pairs))
    return pairs, ntiles


def _host_constants(loc_p, loc_t, pairs, T):
    """CQ [128, 20, T] per core: gathered, K-prescaled pair quantities."""
    Pq = _derived(loc_p, K_SLOPE)
    Tq = _derived(loc_t, K_SLOPE)
    CQs = []
    for c in range(NC):
        n_idx, m_idx = pairs[c]
        cnt = len(n_idx)
        pad = T * 128 - cnt
        if cnt == 0:
            n_idx = np.array([c * NLOC], np.int64)
            m_idx = np.array([0], np.int64)
            cnt, pad = 1, T * 128 - 1
        n_full = np.concatenate([n_idx, np.repeat(n_idx[:1], pad)])
        m_full = np.concatenate([m_idx, np.repeat(m_idx[:1], pad)])
        CQ = np.empty((20, T * 128), dtype=np.float32)
        for qi, q in enumerate(QORD):
            CQ[qi] = Pq[q][n_full]
            CQ[10 + qi] = Tq[q][m_full]
        # [20, T*128] -> [128, 20, T]  (pair j = t*128 + p)
        CQs.append(np.ascontiguousarray(
            CQ.reshape(20, T, 128).transpose(2, 0, 1)).reshape(128, 20 * T))
    return CQs


def _build_nc(T):
    from contextlib import ExitStack

    import concourse.bacc as bacc
    import concourse.tile as tile
    from concourse import mybir
    from concourse.masks import make_identity

    dt = mybir.dt
    op = mybir.AluOpType
    AF = mybir.ActivationFunctionType

    # Bacc (not raw Bass): its finalize() runs generate_event_semaphores,
    # which legalizes Tile's multi-wait sync_info down to <=1 wait per
    # hardware instruction.
    nc = bacc.Bacc(None, target_bir_lowering=False)
    CQ_d = nc.declare_dram_parameter("CQ", [128, 20 * T], dt.float32, isOutput=False)
    BAS_d = nc.declare_dram_parameter("BAS", [128, 4 * NPIX], dt.float16, isOutput=False)
    OUT_d = nc.declare_dram_parameter("OUT", [128, T], dt.float32, isOutput=True)

    with tile.TileContext(nc) as tc, ExitStack() as ctx:
        consts = ctx.enter_context(tc.tile_pool(name="consts", bufs=1))
        work = ctx.enter_context(tc.tile_pool(name="work", bufs=2))
        psum = ctx.enter_context(tc.tile_pool(name="psum", bufs=2, space="PSUM"))

        ident = consts.tile([128, 128], dt.float32)
        make_identity(nc, ident[:])
        CQ = consts.tile([128, 20, T], dt.float32)
        nc.sync.dma_start(out=CQ[:].rearrange("p a b -> p (a b)"), in_=CQ_d[:])
        BAS = consts.tile([128, 4 * NPIX], dt.float16)
        nc.sync.dma_start(out=BAS[:], in_=BAS_d[:])

        def q(i):
            return CQ[:, i, :]

        # ---- coefficient slab C [128 pair, T, 32 r] ----
        # rows r=0..11 = K-scaled affine coefficients (field f rows 3f..3f+2);
        # rows 12..31 junk padding (never fed to the matmul).
        C = consts.tile([128, T, 32], dt.float32)
        S = consts.tile([128, 12, T], dt.float32)
        g = nc.vector

        def s(i):
            return S[:, i, :]

        def c(r):
            return C[:, :, r]

        g.tensor_tensor(s(0), q(0), q(10), op.min)    # k*xmin
        g.tensor_tensor(s(1), q(1), q(11), op.max)    # k*xmax
        g.tensor_tensor(s(2), q(2), q(12), op.min)    # k*ymin
        g.tensor_tensor(s(3), q(3), q(13), op.max)    # k*ymax
        g.tensor_tensor(s(4), s(1), s(0), op.subtract)   # k*sx
        g.tensor_tensor(s(5), s(3), s(2), op.subtract)   # k*sy
        g.tensor_tensor(s(6), s(0), q(4), op.subtract)   # k*dxp
        g.tensor_tensor(s(7), s(2), q(5), op.subtract)   # k*dyp
        # A1 rows 0..2: k*(dxp*ctp + dyp*stp), k*sx*ctp, k*sy*stp
        g.tensor_tensor(s(8), s(6), q(6), op.mult)
        g.tensor_tensor(s(9), s(7), q(7), op.mult)
        g.tensor_tensor(c(0), s(8), s(9), op.add)
        g.tensor_tensor(c(1), s(4), q(6), op.mult)
        g.tensor_tensor(c(2), s(5), q(7), op.mult)
        # B1 rows 6..8: k*(dyp*ctp - dxp*stp), -k*sx*stp, k*sy*ctp
        g.tensor_tensor(s(8), s(7), q(6), op.mult)
        g.tensor_tensor(s(9), s(6), q(7), op.mult)
        g.tensor_tensor(c(6), s(8), s(9), op.subtract)
        g.scalar_tensor_tensor(c(7), s(4), -1.0, q(7), op.mult, op.mult)
        g.tensor_tensor(c(8), s(5), q(6), op.mult)
        # target box offsets
        g.tensor_tensor(s(10), s(0), q(14), op.subtract)  # k*dxt
        g.tensor_tensor(s(11), s(2), q(15), op.subtract)  # k*dyt
        # A2 rows 3..5
        g.tensor_tensor(s(8), s(10), q(16), op.mult)
        g.tensor_tensor(s(9), s(11), q(17), op.mult)
        g.tensor_tensor(c(3), s(8), s(9), op.add)
        g.tensor_tensor(c(4), s(4), q(16), op.mult)
        g.tensor_tensor(c(5), s(5), q(17), op.mult)
        # B2 rows 9..11
        g.tensor_tensor(s(8), s(11), q(16), op.mult)
        g.tensor_tensor(s(9), s(10), q(17), op.mult)
        g.tensor_tensor(c(9), s(8), s(9), op.subtract)
        g.scalar_tensor_tensor(c(10), s(4), -1.0, q(17), op.mult, op.mult)
        g.tensor_tensor(c(11), s(5), q(16), op.mult)

        Ssum = consts.tile([128, T], dt.float32)
        Isum = consts.tile([128, T], dt.float32)

        # ---- main loop over pair tiles ----
        for t in range(T):
            TP = psum.tile([128, 128], dt.float32, tag="tpose", bufs=2)
            if t == 0:
                # Warm the PE on the BAS DMA sem (single-wait) before the
                # first real matmul, which must wait on the ACT-written lhsT.
                # WAW into TP orders the real transpose after it.
                nc.tensor.matmul(
                    TP[:, 0:128], BAS[0:12, 0:128], BAS[0:12, 0:128],
                    start=True, stop=True)
            if True:
                nc.tensor.transpose(TP[0:32, 0:128], C[:, t, :], ident[:])
                lhsT = work.tile([32, 128], dt.float16, tag="lhsT", bufs=2)
                nc.scalar.copy(lhsT[:], TP[0:32, 0:128])

                i = 0
                lw = lhsT[0:12, :]
                F = psum.tile([128, 4 * NPIX], dt.float32, tag="fields", bufs=3)
                for qq in (0, 1):
                    nc.tensor.matmul(
                        F[:, qq * 512:(qq + 1) * 512],
                        lw, BAS[0:12, qq * 512:(qq + 1) * 512],
                        start=True, stop=True)
                # |k*d| : PSUM fp32 -> SBUF fp16 in one DVE pass (abs_max is
                # not ISA-legal in TensorScalar; a size-1-axis reduce with
                # apply_absolute_value is)
                absk = work.tile([128, 4 * NPIX], dt.float16, tag="absk", bufs=2)
                nc.vector.tensor_reduce(
                    absk[:], F[:].rearrange("p (c one) -> p c one", one=1),
                    mybir.AxisListType.X, op.max, apply_absolute_value=True)
                # sigmoid(k*s_f/2 - |k*d|) per field; bias = [128,1] AP
                sig = work.tile([128, 4 * NPIX], dt.float16, tag="sig", bufs=2)
                for f, bq in enumerate((8, 18, 9, 19)):  # khw_p khw_t khh_p khh_t
                    nc.scalar.activation(
                        sig[:, f * NPIX:(f + 1) * NPIX],
                        absk[:, f * NPIX:(f + 1) * NPIX],
                        AF.Sigmoid, bias=CQ[:, bq, t:t + 1], scale=-1.0)
                # Fp = [F1|F2] = [A1,A2]*[B1,B2]; Ssum[t] = sum(F1)+sum(F2)
                # (DVE scalar_tensor_tensor fuses product + pixel sum;
                # InstTensorTensorReduce crashes TRN2 hardware)
                Fp = work.tile([128, 2 * NPIX], dt.float16, tag="Fp", bufs=2)
                nc.vector.scalar_tensor_tensor(
                    Fp[:], sig[:, 0:2 * NPIX], 1.0, sig[:, 2 * NPIX:4 * NPIX],
                    op.mult, op.mult, accum_out=Ssum[:, t:t + 1])
                # F12 = F1*F2 on GPSIMD; Isum[t] = sum(F12) on DVE
                F12 = work.tile([128, NPIX], dt.float16, tag="F12", bufs=2)
                nc.gpsimd.tensor_tensor(
                    F12[:], Fp[:, 0:NPIX], Fp[:, NPIX:2 * NPIX], op.mult)
                nc.vector.tensor_reduce(
                    Isum[:, t:t + 1], F12[:], mybir.AxisListType.X, op.add)

        # ---- epilogue: piou = inter / (stot - inter + eps) ----
        union = consts.tile([128, T], dt.float32)
        nc.vector.scalar_tensor_tensor(
            union[:], Isum[:], -1.0, Ssum[:], op.mult, op.add)
        nc.vector.tensor_scalar(union[:], union[:], float(EPS), None, op.add)
        rec = consts.tile([128, T], dt.float32)
        nc.vector.reciprocal(rec[:], union[:])
        piou = consts.tile([128, T], dt.float32)
        nc.vector.tensor_tensor(piou[:], Isum[:], rec[:], op.mult)
        nc.sync.dma_start(out=OUT_d[:], in_=piou[:])

    nc.finalize()
    return nc


def _get_compiled(T):
    if T not in _cache:
        _cache[T] = _build_nc(T)
    return _cache[T]


def kernel(loc_p, loc_t, grid):
    from concourse.bass_utils import run_bass_kernel_spmd

    assert int(grid) == G
    loc_p = np.asarray(loc_p, dtype=np.float32)
    loc_t = np.asarray(loc_t, dtype=np.float32)
    pairs, T = _active_pairs(loc_p, loc_t)
    CQs = _host_constants(loc_p, loc_t, pairs, T)
    BAS = _basis()

    nc = _get_compiled(T)
    in_maps = [{"CQ": CQs[c], "BAS": BAS} for c in range(NC)]
    res = run_bass_kernel_spmd(nc, in_maps, core_ids=list(range(NC)))
    out = np.zeros((N, M), dtype=np.float32)
    for c in range(NC):
        n_idx, m_idx = pairs[c]
        cnt = len(n_idx)
        vals = res.results[c]["OUT"].T.reshape(-1)[:cnt]  # pair j = t*128+p
        out[n_idx, m_idx] = vals
    return out


# revision 12
# speedup vs baseline: 1.3208x; 1.0802x over previous
"""PIoU (pixel-wise IoU) pairwise matrix kernel for Trainium2, 8 NeuronCores.

Math: for each pair (predicted box n, target box m) the reference samples a
16x16 grid of the joint AABB and evaluates a soft membership
F = sigmoid(k(w/2-|A|)) * sigmoid(k(h/2-|B|)) per box, where (A, B) are the
pixel offsets rotated into the box frame.  A and B are *affine* in the grid
coordinates (ug, uh), so k*(A, B) for all 256 pixels x 4 fields comes from
ONE K=12 matmul per 128-pair tile against a constant [1, ug, uh] basis;
the k*s/2 offset folds into the sigmoid's per-partition bias:
    F-factor = sigmoid(-|k*d| + k*s/2)

NMS gating (the big one): with k=10 the sigmoid tails die within ~1px, so
any pair whose dilated AABBs (delta=2px) do not overlap has
piou < 1e-14 -- indistinguishable from 0 at fp32.  Only ~8% of the 512x512
pairs survive.  The host computes the O(N*M) AABB overlap mask (cheap
numpy), gathers the active pairs into 128-wide tiles (partition = pair),
and scatters the device results back into the zero matrix.  Each core gets
the active pairs of its 64 predicted boxes (~21 tiles); all cores are
padded to the same tile count T with duplicate pairs so one SPMD program
serves all 8.  The 12 affine coefficients per pair are tiny O(active)
host work and ship pre-transposed as the matmul's stationary operand, so
the device runs no coefficient build / transpose / copy at all.

Per 128-pair tile:
  PE      2 fp16 matmuls (512 cols) -> k*d in [128,1024] fp32 PSUM
  DVE     |k*d| via size-1-axis reduce with apply_absolute_value
          (PSUM->SBUF fp16), then fused product+pixel-sum
          scalar_tensor_tensor -> Ssum, and the Isum reduce
  ACT     4x sigmoid(scale=-1, bias=k*s_f/2), bias = [128,1] AP
  GPSIMD  F12 = F1*F2 product
Ssum/Isum land in column t of [128,T] accumulators; the epilogue computes
piou = I/(S-I+eps) for all tiles and DMAs one [128,T] block out.
"""

import numpy as np

N = 512
M = 512
G = 16
NPIX = G * G
K_SLOPE = np.float32(10.0)
EPS = np.float32(1e-6)
NC = 8
NLOC = N // NC  # 64 predicted boxes per core
DELTA = np.float32(2.0)  # AABB dilation for the gating mask (px)

_cache = {}


def _derived(b, k):
    # b: [K,5] float32 -> per-box derived quantities, coords pre-scaled by k
    cx, cy, w, h, t = (b[:, i].astype(np.float32) for i in range(5))
    c, s = np.cos(t).astype(np.float32), np.sin(t).astype(np.float32)
    hw = np.float32(0.5) * (w * np.abs(c) + h * np.abs(s))
    hh = np.float32(0.5) * (w * np.abs(s) + h * np.abs(c))
    return dict(
        cx=k * cx, cy=k * cy, ct=c, st=s,
        khw=(k * np.float32(0.5)) * w, khh=(k * np.float32(0.5)) * h,
        x0=k * (cx - hw), x1=k * (cx + hw), y0=k * (cy - hh), y1=k * (cy + hh),
    )


def _basis():
    u = ((np.arange(G, dtype=np.float32) + np.float32(0.5)) / np.float32(G))
    Ug = np.tile(u, G)      # pixel p = h*G+g -> u[g]
    Uh = np.repeat(u, G)    # -> u[h]
    # [12, 1024]: field f (A1,A2,B1,B2) rows 3f..3f+2 = 1,Ug,Uh on its cols
    basis = np.zeros((12, 4 * NPIX), dtype=np.float16)
    for f in range(4):
        c0 = f * NPIX
        basis[3 * f + 0, c0:c0 + NPIX] = 1.0
        basis[3 * f + 1, c0:c0 + NPIX] = Ug.astype(np.float16)
        basis[3 * f + 2, c0:c0 + NPIX] = Uh.astype(np.float16)
    return basis


def _active_pairs(loc_p, loc_t):
    """Per-core gathered pair lists from the dilated-AABB overlap mask."""
    P = _derived(loc_p, np.float32(1.0))
    T = _derived(loc_t, np.float32(1.0))
    ov = ((np.minimum(P["x1"][:, None], T["x1"][None, :]) + DELTA
           >= np.maximum(P["x0"][:, None], T["x0"][None, :])) &
          (np.minimum(P["y1"][:, None], T["y1"][None, :]) + DELTA
           >= np.maximum(P["y0"][:, None], T["y0"][None, :])))
    pairs = []
    for c in range(NC):
        n_idx, m_idx = np.nonzero(ov[c * NLOC:(c + 1) * NLOC])
        pairs.append((n_idx.astype(np.int64) + c * NLOC, m_idx.astype(np.int64)))
    ntiles = max(1, max((len(n) + 127) // 128 for n, _ in pairs))
    return pairs, ntiles


def _host_constants(loc_p, loc_t, pairs, T):
    """Per-core LH [12, T*128] fp16 (pre-transposed matmul coefficients) and
    BI [128, 4*T] fp32 (sigmoid biases k*s_f/2)."""
    Pq = _derived(loc_p, K_SLOPE)
    Tq = _derived(loc_t, K_SLOPE)
    LHs, BIs = [], []
    for c in range(NC):
        n_idx, m_idx = pairs[c]
        cnt = len(n_idx)
        if cnt == 0:
            n_idx = np.array([c * NLOC], np.int64)
            m_idx = np.array([0], np.int64)
            cnt = 1
        pad = T * 128 - cnt
        n = np.concatenate([n_idx, np.repeat(n_idx[:1], pad)])
        m = np.concatenate([m_idx, np.repeat(m_idx[:1], pad)])

        xmin = np.minimum(Pq["x0"][n], Tq["x0"][m])
        xmax = np.maximum(Pq["x1"][n], Tq["x1"][m])
        ymin = np.minimum(Pq["y0"][n], Tq["y0"][m])
        ymax = np.maximum(Pq["y1"][n], Tq["y1"][m])
        sx, sy = xmax - xmin, ymax - ymin
        dxp, dyp = xmin - Pq["cx"][n], ymin - Pq["cy"][n]
        dxt, dyt = xmin - Tq["cx"][m], ymin - Tq["cy"][m]
        ctp, stp = Pq["ct"][n], Pq["st"][n]
        ctt, stt = Tq["ct"][m], Tq["st"][m]
        LH = np.stack([
            dxp * ctp + dyp * stp, sx * ctp, sy * stp,   # A1
            dxt * ctt + dyt * stt, sx * ctt, sy * stt,   # A2
            dyp * ctp - dxp * stp, -sx * stp, sy * ctp,  # B1
            dyt * ctt - dxt * stt, -sx * stt, sy * ctt,  # B2
        ], axis=0).astype(np.float16)                    # [12, T*128]
        LHs.append(np.ascontiguousarray(LH))
        BI = np.stack([Pq["khw"][n], Tq["khw"][m], Pq["khh"][n], Tq["khh"][m]],
                      axis=0)                            # [4, T*128]
        BIs.append(np.ascontiguousarray(
            BI.reshape(4, T, 128).transpose(2, 0, 1)).reshape(128, 4 * T))
    return LHs, BIs


def _build_nc(T):
    from contextlib import ExitStack

    import concourse.bacc as bacc
    import concourse.tile as tile
    from concourse import mybir

    dt = mybir.dt
    op = mybir.AluOpType
    AF = mybir.ActivationFunctionType

    # Bacc (not raw Bass): its finalize() runs generate_event_semaphores,
    # which legalizes Tile's multi-wait sync_info down to <=1 wait per
    # hardware instruction.
    nc = bacc.Bacc(None, target_bir_lowering=False)
    LH_d = nc.declare_dram_parameter("LH", [12, T * 128], dt.float16, isOutput=False)
    BI_d = nc.declare_dram_parameter("BI", [128, 4 * T], dt.float32, isOutput=False)
    BAS_d = nc.declare_dram_parameter("BAS", [12, 4 * NPIX], dt.float16, isOutput=False)
    OUT_d = nc.declare_dram_parameter("OUT", [128, T], dt.float32, isOutput=True)

    with tile.TileContext(nc) as tc, ExitStack() as ctx:
        consts = ctx.enter_context(tc.tile_pool(name="consts", bufs=1))
        work = ctx.enter_context(tc.tile_pool(name="work", bufs=2))
        psum = ctx.enter_context(tc.tile_pool(name="psum", bufs=2, space="PSUM"))

        LH = consts.tile([12, T, 128], dt.float16)
        nc.sync.dma_start(out=LH[:].rearrange("p a b -> p (a b)"), in_=LH_d[:])
        BI = consts.tile([128, 4, T], dt.float32)
        nc.sync.dma_start(out=BI[:].rearrange("p a b -> p (a b)"), in_=BI_d[:])
        BAS = consts.tile([12, 4 * NPIX], dt.float16)
        nc.sync.dma_start(out=BAS[:], in_=BAS_d[:])

        Ssum = consts.tile([128, T], dt.float32)
        Isum = consts.tile([128, T], dt.float32)
        warm = consts.tile([128, 1], dt.float32)

        # ---- main loop over pair tiles ----
        for t in range(T):
            F = psum.tile([128, 4 * NPIX], dt.float32, tag="fields", bufs=4)
            if t == 0:
                # Each engine's first real op may only carry one HW sem wait;
                # absorb one of its two input sems with a dummy op first.
                # PE: warm on the BAS DMA (real MM then waits only LH DMA).
                nc.tensor.matmul(
                    F[:, 0:128], BAS[0:12, 0:128], BAS[0:12, 0:128],
                    start=True, stop=True)
                # ACT: warm on the BI DMA (first sigmoid then waits only DVE);
                # also pulls the Sigmoid table load off the critical path.
                nc.scalar.activation(warm[:], BI[:, 0, 0:1], AF.Sigmoid)
            for qq in (0, 1):
                nc.tensor.matmul(
                    F[:, qq * 512:(qq + 1) * 512],
                    LH[:, t, :], BAS[:, qq * 512:(qq + 1) * 512],
                    start=True, stop=True)
            # |k*d| : PSUM fp32 -> SBUF fp16 in one DVE pass (abs_max is not
            # ISA-legal in TensorScalar; a size-1-axis reduce with
            # apply_absolute_value is)
            absk = work.tile([128, 4 * NPIX], dt.float16, tag="absk", bufs=2)
            nc.vector.tensor_reduce(
                absk[:], F[:].rearrange("p (c one) -> p c one", one=1),
                mybir.AxisListType.X, op.max, apply_absolute_value=True)
            # sigmoid(k*s_f/2 - |k*d|) per field; bias = [128,1] AP
            sig = work.tile([128, 4 * NPIX], dt.float16, tag="sig", bufs=2)
            for f in range(4):
                nc.scalar.activation(
                    sig[:, f * NPIX:(f + 1) * NPIX],
                    absk[:, f * NPIX:(f + 1) * NPIX],
                    AF.Sigmoid, bias=BI[:, f, t:t + 1], scale=-1.0)
            # Fp = [F1|F2] = [A1,A2]*[B1,B2]; Ssum[t] = sum(F1)+sum(F2)
            # (DVE scalar_tensor_tensor fuses product + pixel sum;
            # InstTensorTensorReduce crashes TRN2 hardware)
            Fp = work.tile([128, 2 * NPIX], dt.float16, tag="Fp", bufs=2)
            nc.vector.scalar_tensor_tensor(
                Fp[:], sig[:, 0:2 * NPIX], 1.0, sig[:, 2 * NPIX:4 * NPIX],
                op.mult, op.mult, accum_out=Ssum[:, t:t + 1])
            # F12 = F1*F2 on GPSIMD; Isum[t] = sum(F12) on DVE
            F12 = work.tile([128, NPIX], dt.float16, tag="F12", bufs=2)
            nc.gpsimd.tensor_tensor(
                F12[:], Fp[:, 0:NPIX], Fp[:, NPIX:2 * NPIX], op.mult)
            nc.vector.tensor_reduce(
                Isum[:, t:t + 1], F12[:], mybir.AxisListType.X, op.add)

        # ---- epilogue: piou = inter / (stot - inter + eps) ----
        union = consts.tile([128, T], dt.float32)
        nc.vector.scalar_tensor_tensor(
            union[:], Isum[:], -1.0, Ssum[:], op.mult, op.add)
        nc.vector.tensor_scalar(union[:], union[:], float(EPS), None, op.add)
        rec = consts.tile([128, T], dt.float32)
        nc.vector.reciprocal(rec[:], union[:])
        piou = consts.tile([128, T], dt.float32)
        nc.vector.tensor_tensor(piou[:], Isum[:], rec[:], op.mult)
        nc.sync.dma_start(out=OUT_d[:], in_=piou[:])

    nc.finalize()
    return nc


def _get_compiled(T):
    if T not in _cache:
        _cache[T] = _build_nc(T)
    return _cache[T]


def kernel(loc_p, loc_t, grid):
    from concourse.bass_utils import run_bass_kernel_spmd

    assert int(grid) == G
    loc_p = np.asarray(loc_p, dtype=np.float32)
    loc_t = np.asarray(loc_t, dtype=np.float32)
    pairs, T = _active_pairs(loc_p, loc_t)
    LHs, BIs = _host_constants(loc_p, loc_t, pairs, T)
    BAS = _basis()

    nc = _get_compiled(T)
    in_maps = [{"LH": LHs[c], "BI": BIs[c], "BAS": BAS} for c in range(NC)]
    res = run_bass_kernel_spmd(nc, in_maps, core_ids=list(range(NC)))
    out = np.zeros((N, M), dtype=np.float32)
    for c in range(NC):
        n_idx, m_idx = pairs[c]
        cnt = len(n_idx)
        vals = res.results[c]["OUT"].T.reshape(-1)[:cnt]  # pair j = t*128+p
        out[n_idx, m_idx] = vals
    return out
